# revision 1
# baseline (speedup 1.0000x reference)
"""LoRA Linear layer on 8 Trainium2 NeuronCores.

Computes out = x @ W.T + bias + scaling * (x @ A.T) @ B.T for
x [4, 4096, 4096] f32, W [4096, 4096], bias [4096], A [16, 4096], B [4096, 16].

Strategy:
- Host: fold the rank-16 LoRA path into the weight (exact up to f32
  rounding): W_eff = W.T + scaling * (A.T @ B.T), layout [in, out], fp16.
- Shard data-parallel over the batch: 16384 rows of x split 8 x 2048.
  W_eff replicated per core; no collectives. bias is added on the host
  in f32 (removes the bias input and its SBUF/DMA cost entirely).
- Per core: out_s[2048, 4096] = x_s @ W_eff as fp16 matmuls with fp32
  PSUM accumulation (fro rel err ~3e-4 vs f32 reference).
- x is kept SBUF-resident per half (2 halves of 8 m-tiles, [128,32,128]
  fp16 = 64KB/partition): x HBM traffic is 16.8MB total instead of the
  67MB a stream-x-per-W-block structure pays.
- W streams once per half (2 x 33.5MB) through a 64-chunk (2-block)
  rolling window, so block b+1's chunks stream during block b's compute.
- Output fp16 (16.8MB instead of 33.5MB f32); host upcasts to f32.
- The first block of the first half runs 4 m-tiles interleaved across all
  8 PSUM banks so the cold W-block-0 DMA stream is absorbed by compute
  (and the PE HAM clock-gate warms) instead of idling the PE.
- DMA queues: W on sync (HWDGE), x on gpsimd (SWDGE), out on scalar
  (HWDGE) so slot-wait head-of-line blocking can't cross streams.
Per-core DMA ~100MB/pass vs ~136MB for the stream-x baseline.
"""

import numpy as np

IN_F = 4096
OUT_F = 4096
R = 16
SCALING = 32.0 / R
N_CORES = 8
M_TOTAL = 4 * 4096
M_CORE = M_TOTAL // N_CORES  # 2048

P = 128
KO = IN_F // P  # 32 contraction chunks
NW = 512  # matmul free dim (one PSUM bank of f32)
NJ = 2  # n-tiles per block (stationary reused NJ times)
NB = OUT_F // (NJ * NW)  # 4 n blocks
NBW = NJ * NW  # 1024 cols per block
MT = M_CORE // P  # 16 m tiles
HALVES = 2
MH = MT // HALVES  # 8 m tiles resident per half

_CACHE = {}


def _build_nc(repeats=1):
    """repeats>1 replays the whole compute pass (W/x re-streamed) — used
    only for device-time measurement by test.py."""
    import concourse.mybir as mybir
    import concourse.tile as tile
    from concourse import bacc

    nc = bacc.Bacc("TRN2", target_bir_lowering=False, debug=False,
                   num_devices=N_CORES)
    xk = nc.dram_tensor("xk", [MT, P, KO, P], mybir.dt.float16,
                        kind="ExternalInput").ap()
    w = nc.dram_tensor("w", [IN_F, OUT_F], mybir.dt.float16,
                       kind="ExternalInput").ap()
    out = nc.dram_tensor("out", [M_CORE, OUT_F], mybir.dt.float16,
                         kind="ExternalOutput").ap()

    # 2-ko batched W tiles: wr2[kp, p, k, n] = w[kp*256 + k*128 + p, n]
    wr2 = w.rearrange("(kp k p) n -> kp p k n", k=2, p=P)

    with tile.TileContext(nc) as tc:
        with (
            tc.tile_pool(name="xpool", bufs=MH) as xpool,
            tc.tile_pool(name="wpool", bufs=32) as wpool,
            tc.tile_pool(name="opool", bufs=4) as opool,
            tc.tile_pool(name="pspool", bufs=8, space="PSUM") as pspool,
        ):
            for rep in range(repeats):
                for h in range(HALVES):
                    xt = {}
                    for i in range(MH):
                        t = xpool.tile([P, KO, P], mybir.dt.float16,
                                       name=f"x{rep}_{h}_{i}", tag="x",
                                       bufs=MH)
                        nc.gpsimd.dma_start(t[:], xk[h * MH + i])
                        xt[i] = t
                    for nb in range(NB):
                        wts = []
                        for kp in range(KO // 2):
                            wt_ = wpool.tile([P, 2, NBW], mybir.dt.float16,
                                             name=f"w{rep}_{h}_{nb}_{kp}",
                                             tag="w", bufs=32)
                            nc.sync.dma_start(
                                wt_[:],
                                wr2[kp, :, :, nb * NBW:(nb + 1) * NBW])
                            wts.append(wt_)
                        # wide start: absorb the cold W stream with 4
                        # interleaved m-tiles on the very first block
                        first = (rep == 0 and h == 0 and nb == 0)
                        i = 0
                        while i < MH:
                            g = 4 if (first and i == 0) else 1
                            idxs = list(range(i, i + g))
                            pss = {}
                            for j in idxs:
                                for nj in range(NJ):
                                    pss[(j, nj)] = pspool.tile(
                                        [P, NW], mybir.dt.float32,
                                        name=f"ps{rep}_{h}_{nb}_{j}_{nj}",
                                        tag="ps")
                            for ko in range(KO):
                                for j in idxs:
                                    lhsT = xt[j][:, ko, :]
                                    for nj in range(NJ):
                                        nc.tensor.matmul(
                                            pss[(j, nj)][:],
                                            lhsT,
                                            wts[ko // 2][
                                                :, ko % 2,
                                                nj * NW:(nj + 1) * NW],
                                            start=(ko == 0),
                                            stop=(ko == KO - 1),
                                        )
                            for j in idxs:
                                ot = opool.tile([P, NBW], mybir.dt.float16,
                                                name=f"o{rep}_{h}_{nb}_{j}",
                                                tag="o", bufs=4)
                                for nj in range(NJ):
                                    nc.vector.tensor_copy(
                                        ot[:, nj * NW:(nj + 1) * NW],
                                        pss[(j, nj)][:])
                                m0 = (h * MH + j) * P
                                nc.scalar.dma_start(
                                    out[m0:m0 + P, nb * NBW:(nb + 1) * NBW],
                                    ot[:])
                            i += g

    nc.compile()
    return nc


def _get_nc():
    if "nc" not in _CACHE:
        _CACHE["nc"] = _build_nc()
    return _CACHE["nc"]


def make_in_maps(x, weight, bias, lora_A, lora_B):
    """Host-side shard prep: returns the per-core input maps."""
    w_eff = weight.T.astype(np.float32) + np.float32(SCALING) * (
        lora_A.T.astype(np.float32) @ lora_B.T.astype(np.float32))
    w16 = w_eff.astype(np.float16)
    xf = np.asarray(x, dtype=np.float32).reshape(M_TOTAL, IN_F)
    in_maps = []
    for c in range(N_CORES):
        xs = xf[c * M_CORE:(c + 1) * M_CORE]
        xT = np.ascontiguousarray(xs.T, dtype=np.float16)  # [IN_F, M_CORE]
        # pack to [m_tile, p, ko, m] so each m-tile is one contiguous DMA
        xk = np.ascontiguousarray(
            xT.reshape(KO, P, MT, P).transpose(2, 1, 0, 3))
        in_maps.append({"xk": xk, "w": w16})
    return in_maps


def kernel(x, weight, bias, lora_A, lora_B):
    from concourse.bass_utils import run_bass_kernel_spmd

    nc = _get_nc()
    in_maps = make_in_maps(x, weight, bias, lora_A, lora_B)
    res = run_bass_kernel_spmd(nc, in_maps, core_ids=list(range(N_CORES)))
    _CACHE["last_result"] = res
    out = np.concatenate([r["out"] for r in res.results], axis=0)
    out = out.astype(np.float32) + np.asarray(bias, np.float32)
    return out.reshape(4, 4096, OUT_F)



# revision 4
# speedup vs baseline: 1.3349x; 1.3349x over previous
"""LoRA Linear layer on 8 Trainium2 NeuronCores.

Computes out = x @ W.T + bias + scaling * (x @ A.T) @ B.T for
x [4, 4096, 4096] f32, W [4096, 4096], bias [4096], A [16, 4096], B [4096, 16].

Strategy:
- Host: fold the rank-16 LoRA path into the weight (exact up to f32
  rounding): W_eff = W.T + scaling * (A.T @ B.T), layout [in, out], fp16.
- Shard data-parallel over the batch: 16384 rows of x split 8 x 2048.
  W_eff replicated per core; no collectives. bias is added on the host
  in f32 (removes the bias input and its SBUF/DMA cost entirely).
- Per core: out_s[2048, 4096] = x_s @ W_eff as fp16 matmuls with fp32
  PSUM accumulation (fro rel err ~3e-4 vs f32 reference).
- x is kept SBUF-resident per half (2 halves of 8 m-tiles, [128,32,128]
  fp16 = 64KB/partition): x HBM traffic is 16.8MB total instead of the
  67MB a stream-x-per-W-block structure pays.
- W streams once per half (2 x 33.5MB) through a 64-chunk (2-block)
  rolling window, so block b+1's chunks stream during block b's compute.
- Output fp16 (16.8MB instead of 33.5MB f32); host upcasts to f32.
- The first block of the first half runs 4 m-tiles interleaved across all
  8 PSUM banks so the cold W-block-0 DMA stream is absorbed by compute
  (and the PE HAM clock-gate warms) instead of idling the PE.
- DMA queues: W on sync (HWDGE), x on gpsimd (SWDGE), out on scalar
  (HWDGE) so slot-wait head-of-line blocking can't cross streams.
Per-core DMA ~100MB/pass vs ~136MB for the stream-x baseline.

Measured limits (see memory notes): this kernel sits at the 8-core
data-dependent power roofline (~1.96 GHz effective PE clock for dense
random-data fp16 GEMM; 4096 MMs x 512 cyc ≈ 1070 us/pass). One core
alone runs the same MM stream at the full 2.4 GHz (874 us-equivalent),
so the gap is chip power, not scheduling. Zero-data probes show the
throttle is dominated by the result-entropy path (PSUM/evict/out),
which is irreducible; operand mantissa truncation (W_MANTISSA_BITS)
changes the clock only within noise, so it stays at 10 (exact fp16).
"""

import numpy as np

IN_F = 4096
OUT_F = 4096
R = 16
SCALING = 32.0 / R
N_CORES = 8
M_TOTAL = 4 * 4096
M_CORE = M_TOTAL // N_CORES  # 2048

P = 128
KO = IN_F // P  # 32 contraction chunks
NW = 512  # matmul free dim (one PSUM bank of f32)
NJ = 2  # n-tiles per block (stationary reused NJ times)
NB = OUT_F // (NJ * NW)  # 4 n blocks
NBW = NJ * NW  # 1024 cols per block
MT = M_CORE // P  # 16 m tiles
HALVES = 2
MH = MT // HALVES  # 8 m tiles resident per half

_CACHE = {}

# Mantissa bits kept in the fp16 W (moving matmul operand). The 8-core PE
# power throttle responds to active mantissa bits of the moving stream;
# truncating W's mantissa raises the sustained PE clock. 10 = full fp16.
W_MANTISSA_BITS = 10


def _trunc_fp16(a, mbits):
    """Round fp16 mantissa to mbits (carry into exponent is correct)."""
    u = np.ascontiguousarray(a, dtype=np.float16).view(np.uint16)
    drop = 10 - mbits
    if drop <= 0:
        return a.astype(np.float16)
    half = np.uint16(1 << (drop - 1))
    u = (u + half) & np.uint16((~((1 << drop) - 1)) & 0xFFFF)
    return u.view(np.float16)


def _build_nc(repeats=1):
    """repeats>1 replays the whole compute pass (W/x re-streamed) — used
    only for device-time measurement by test.py."""
    import concourse.mybir as mybir
    import concourse.tile as tile
    from concourse import bacc

    nc = bacc.Bacc("TRN2", target_bir_lowering=False, debug=False,
                   num_devices=N_CORES)
    xk = nc.dram_tensor("xk", [MT, P, KO, P], mybir.dt.float16,
                        kind="ExternalInput").ap()
    w = nc.dram_tensor("w", [IN_F, OUT_F], mybir.dt.float16,
                       kind="ExternalInput").ap()
    out = nc.dram_tensor("out", [M_CORE, OUT_F], mybir.dt.float16,
                         kind="ExternalOutput").ap()

    # 2-ko batched W tiles: wr2[kp, p, k, n] = w[kp*256 + k*128 + p, n]
    wr2 = w.rearrange("(kp k p) n -> kp p k n", k=2, p=P)

    with tile.TileContext(nc) as tc:
        with (
            tc.tile_pool(name="xpool", bufs=MH) as xpool,
            tc.tile_pool(name="wpool", bufs=32) as wpool,
            tc.tile_pool(name="opool", bufs=4) as opool,
            tc.tile_pool(name="pspool", bufs=8, space="PSUM") as pspool,
        ):
            for rep in range(repeats):
                for h in range(HALVES):
                    xt = {}
                    for i in range(MH):
                        t = xpool.tile([P, KO, P], mybir.dt.float16,
                                       name=f"x{rep}_{h}_{i}", tag="x",
                                       bufs=MH)
                        nc.gpsimd.dma_start(t[:], xk[h * MH + i])
                        xt[i] = t
                    for nb in range(NB):
                        wts = []
                        for kp in range(KO // 2):
                            wt_ = wpool.tile([P, 2, NBW], mybir.dt.float16,
                                             name=f"w{rep}_{h}_{nb}_{kp}",
                                             tag="w", bufs=32)
                            nc.sync.dma_start(
                                wt_[:],
                                wr2[kp, :, :, nb * NBW:(nb + 1) * NBW])
                            wts.append(wt_)
                        # wide start: absorb the cold W stream with 4
                        # interleaved m-tiles on the very first block
                        first = (rep == 0 and h == 0 and nb == 0)
                        i = 0
                        while i < MH:
                            g = 4 if (first and i == 0) else 1
                            idxs = list(range(i, i + g))
                            pss = {}
                            for j in idxs:
                                for nj in range(NJ):
                                    pss[(j, nj)] = pspool.tile(
                                        [P, NW], mybir.dt.float32,
                                        name=f"ps{rep}_{h}_{nb}_{j}_{nj}",
                                        tag="ps")
                            for ko in range(KO):
                                for j in idxs:
                                    lhsT = xt[j][:, ko, :]
                                    for nj in range(NJ):
                                        nc.tensor.matmul(
                                            pss[(j, nj)][:],
                                            lhsT,
                                            wts[ko // 2][
                                                :, ko % 2,
                                                nj * NW:(nj + 1) * NW],
                                            start=(ko == 0),
                                            stop=(ko == KO - 1),
                                        )
                            for j in idxs:
                                ot = opool.tile([P, NBW], mybir.dt.float16,
                                                name=f"o{rep}_{h}_{nb}_{j}",
                                                tag="o", bufs=4)
                                for nj in range(NJ):
                                    nc.vector.tensor_copy(
                                        ot[:, nj * NW:(nj + 1) * NW],
                                        pss[(j, nj)][:])
                                m0 = (h * MH + j) * P
                                nc.scalar.dma_start(
                                    out[m0:m0 + P, nb * NBW:(nb + 1) * NBW],
                                    ot[:])
                            i += g

    nc.compile()
    return nc


def _get_nc():
    if "nc" not in _CACHE:
        _CACHE["nc"] = _build_nc()
    return _CACHE["nc"]


def make_in_maps(x, weight, bias, lora_A, lora_B):
    """Host-side shard prep: returns the per-core input maps."""
    w_eff = weight.T.astype(np.float32) + np.float32(SCALING) * (
        lora_A.T.astype(np.float32) @ lora_B.T.astype(np.float32))
    w16 = _trunc_fp16(w_eff, W_MANTISSA_BITS)
    xf = np.asarray(x, dtype=np.float32).reshape(M_TOTAL, IN_F)
    in_maps = []
    for c in range(N_CORES):
        xs = xf[c * M_CORE:(c + 1) * M_CORE]
        xT = np.ascontiguousarray(xs.T, dtype=np.float16)  # [IN_F, M_CORE]
        # pack to [m_tile, p, ko, m] so each m-tile is one contiguous DMA
        xk = np.ascontiguousarray(
            xT.reshape(KO, P, MT, P).transpose(2, 1, 0, 3))
        in_maps.append({"xk": xk, "w": w16})
    return in_maps


def kernel(x, weight, bias, lora_A, lora_B):
    from concourse.bass_utils import run_bass_kernel_spmd

    nc = _get_nc()
    in_maps = make_in_maps(x, weight, bias, lora_A, lora_B)
    res = run_bass_kernel_spmd(nc, in_maps, core_ids=list(range(N_CORES)))
    _CACHE["last_result"] = res
    out = np.concatenate([r["out"] for r in res.results], axis=0)
    out = out.astype(np.float32) + np.asarray(bias, np.float32)
    return out.reshape(4, 4096, OUT_F)

